# revision 35
# baseline (speedup 1.0000x reference)
"""Distributed Trainium2 Bass kernel for nn_Attention_74732430950409.

Single-query MHA with RoPE'd keys/values. All big GEMMs are folded onto the
tiny query/head side:

  qtil[h,:] = (((x @ Wq.T) @ Wq_mha.T)[h] @ Wk_mha[h]) @ Wk        (16, 2048)
  logits[s,h] = rope(keys)[s,:] . qtil[h,:] / sqrt(128)
  w = exp(logits)          (no max subtraction; |logits| < ~7)
  u[h,:] = sum_s w[s,h] * rope(states)[s,:]                        (16, 2048)
  z[h,:]  = (u[h,:] @ Wv.T) / l[h]
  attn[h,:] = z[h,:] @ Wv_mha[h].T
  out = attn.flat @ Wo.T + x

Timing model (measured): the 8 SPMD cores launch ~60us apart, so the first
collective is a rendezvous that dominates the front half. Everything before
it (bulk DMA, RoPE on DVE, q-path) is effectively free; the optimization
target is the post-rendezvous serial chain AR(qh) -> AR(qtil) -> logits ->
u -> AR(u|l) -> epilogue -> AG(attn) -> out. Hence:
  * classic 6-op RoPE on DVE (hidden pre-rendezvous) to halve the
    post-rendezvous PE matmul count,
  * fp8(e4m3, x16-prescaled) weights; descales folded into activation
    scales,
  * merged DMA transfers (the sync sequencer pays ~0.6us dispatch per DMA),
  * collective bounces on the gpsimd SWDGE ring (not FIFO-blocked behind
    bulk HBM traffic),
  * bf16 collective payloads; final collective is an AllGather + local sum
    (AG floor ~4.6us vs AR ~10us),
  * PSUM->SBUF copies bundled into few wide activations.
"""

import sys
import numpy as np

for p in ("/opt/trn_rl_repo",):
    if p not in sys.path:
        sys.path.insert(0, p)

import ml_dtypes

BF16 = ml_dtypes.bfloat16
FP8 = ml_dtypes.float8_e4m3fn

NUM_HEADS = 16
QK = 2048
VO = 2048
S = 8192
NC = 8
S_LOC = S // NC          # 1024
SH = VO // NC            # 256 rows per core of each weight
DQ = QK // NUM_HEADS     # 128
HALF = VO // 2           # 1024
ROPE_THETA = 10000.0
WSCALE = 16.0            # fp8 weight pre-scale (keeps values out of subnormals)

_cache = {}


def _build():
    import concourse.bass as bass
    import concourse.mybir as mybir
    import concourse.bacc as bacc
    import concourse.tile as tile

    f32 = mybir.dt.float32
    bf16 = mybir.dt.bfloat16
    fp8 = mybir.dt.float8e4
    AF = mybir.ActivationFunctionType
    ALU = mybir.AluOpType
    PSUM = bass.MemorySpace.PSUM

    nc = bacc.Bacc(None, target_bir_lowering=False)

    # ---------------- DRAM parameters (host pre-swizzled; contiguous) ------
    xq_d = nc.dram_tensor("xq", [128, 16], bf16, kind="ExternalInput")
    identb_d = nc.dram_tensor("identb", [128, 128], bf16, kind="ExternalInput")
    identf_d = nc.dram_tensor("identf", [16, 16], f32, kind="ExternalInput")
    xo_d = nc.dram_tensor("xo", [1, SH], f32, kind="ExternalInput")

    wqa_d = nc.dram_tensor("wqa", [128, 2, 4096], fp8, kind="ExternalInput")
    wqb_d = nc.dram_tensor("wqb", [128, 2, 4096], fp8, kind="ExternalInput")
    wep_d = nc.dram_tensor("wep", [128, 3, 4096], fp8, kind="ExternalInput")

    kab_d = nc.dram_tensor("kab", [8, 128, 2, S_LOC], bf16, kind="ExternalInput")
    cksk_d = nc.dram_tensor("cksk", [8, 128, 2, S_LOC], bf16, kind="ExternalInput")
    st_d = nc.dram_tensor("st", [4, 128, 2, VO], bf16, kind="ExternalInput")
    csss_d = nc.dram_tensor("csss", [8, 128, 2, HALF], bf16, kind="ExternalInput")

    out_d = nc.dram_tensor("out", [1, SH], f32, kind="ExternalOutput")

    RG = [list(range(NC))]
    # qtil is rescaled to ~fp8 range at the AR#2 copy (x16 / x16^4), leaving
    # one factor of 16 for the exp scale
    SCALE_EXP = float(1.0 / np.sqrt(DQ) / WSCALE)
    SCALE_QTP = float(WSCALE / (WSCALE ** 4))

    with tile.TileContext(nc) as tc:
        with (
            tc.tile_pool(name="wts", bufs=1) as wts,
            tc.tile_pool(name="kbuf", bufs=1) as kbuf,
            tc.tile_pool(name="sbuf_s", bufs=1) as sbuf_s,
            tc.tile_pool(name="tmps", bufs=1) as tmps,
            tc.tile_pool(name="small", bufs=1) as small,
            tc.tile_pool(name="psA", bufs=4, space=PSUM) as psA,
            tc.tile_pool(name="psB", bufs=4, space=PSUM) as psB,
            tc.tile_pool(name="dram", bufs=1, space="DRAM") as dram,
        ):
            # ---------------- collective bounce buffers (DRAM) ----------------
            bqh_in = dram.tile([128, NUM_HEADS], bf16)
            bqh_out = dram.tile([128, NUM_HEADS], bf16)
            bqt_in = [dram.tile([128, 8 * NUM_HEADS], fp8, name=f"bqt_in{g}")
                      for g in range(2)]
            bqt_out = [dram.tile([128, 8 * NUM_HEADS], fp8, name=f"bqt_out{g}")
                       for g in range(2)]
            bu_in = dram.tile([128, 16 * NUM_HEADS + 1], bf16)
            bu_out = dram.tile([128, 16 * NUM_HEADS + 1], bf16)
            bat_in = dram.tile([NUM_HEADS, 128], bf16)
            bat_out = dram.tile([128, 128], bf16)

            # ---------------- small persistent SBUF tiles ----------------
            x_sb = small.tile([128, 16], bf16, tag="x")
            identb = small.tile([128, 128], bf16, tag="idb")
            identf = small.tile([16, 16], f32, tag="idf")
            xo_sb = small.tile([1, SH], f32, tag="xo")
            qT_sb = small.tile([128, 2], bf16, tag="qT")
            qhTp_sb = small.tile([128, NUM_HEADS], bf16, tag="qhTp")
            qhT_sb = small.tile([128, NUM_HEADS], bf16, tag="qhT")
            tmpT_sb = small.tile([128, 2, NUM_HEADS], bf16, tag="tmpT")
            qtp_sb = small.tile([128, 16 * NUM_HEADS], fp8, tag="qtp")
            qtilT8 = [small.tile([128, 4, 2, NUM_HEADS], fp8, tag=f"qtilT8_{g}",
                                 name=f"qtilT8_{g}") for g in range(2)]
            w_sb = small.tile([NUM_HEADS, S_LOC], bf16, tag="w")
            l0_sb = small.tile([NUM_HEADS, 1], f32, tag="l0")
            l1_sb = small.tile([NUM_HEADS, 1], f32, tag="l1")
            lp_sb = small.tile([NUM_HEADS, 1], f32, tag="lp")
            wT4_sb = small.tile([128, 4, 2, NUM_HEADS], fp8, tag="wT4")
            u_sb = small.tile([NUM_HEADS, VO], f32, tag="u")
            ub_sb = small.tile([128, 16 * NUM_HEADS + 1], bf16, tag="ub")
            uT_bf = small.tile([128, 16, NUM_HEADS], bf16, tag="uTb")
            l_sb = small.tile([NUM_HEADS, 1], bf16, tag="l")
            l16_sb = small.tile([NUM_HEADS, 1], f32, tag="l16")
            rl_sb = small.tile([NUM_HEADS, 1], f32, tag="rl")
            z_sb = small.tile([NUM_HEADS, SH], bf16, tag="z")
            zT_sb = small.tile([128, 2, NUM_HEADS], bf16, tag="zT")
            atT_sb = small.tile([128, NUM_HEADS], bf16, tag="atT")
            atr_sb = small.tile([NUM_HEADS, 128], bf16, tag="atr")
            aga_sb = small.tile([128, 128], bf16, tag="aga")
            agat_sb = small.tile([128, 128], bf16, tag="agat")
            atT_bf = small.tile([128, NUM_HEADS], bf16, tag="atTb")
            out_sb = small.tile([1, SH], f32, tag="out")

            # ================= DMA issue order (sync HWDGE ring) ===========
            # x + q-projection weights first: they gate the AR#1 trigger,
            # whose latest arrival across cores pins the rendezvous.
            wqa_sb = wts.tile([128, 2, 4096], fp8, tag="wqa")
            wqb_sb = wts.tile([128, 2, 4096], fp8, tag="wqb")
            nc.sync.dma_start(x_sb[:], xq_d[:, :])
            nc.sync.dma_start(wqa_sb[:], wqa_d[:, :, :])
            nc.sync.dma_start(identb[:], identb_d[:, :])
            nc.sync.dma_start(identf[:], identf_d[:, :])
            nc.sync.dma_start(xo_sb[:], xo_d[:, :])

            kab_t, cksk_t = [], []
            for ci in range(8):
                kab = kbuf.tile([128, 2, S_LOC], bf16, tag="kab", bufs=8, name=f"kab{ci}")
                cksk = kbuf.tile([128, 2, S_LOC], bf16, tag="cksk", bufs=4,
                                 name=f"cksk{ci}")
                nc.sync.dma_start(kab[:], kab_d[ci, :, :, :])
                nc.sync.dma_start(cksk[:], cksk_d[ci, :, :, :])
                kab_t.append(kab); cksk_t.append(cksk)

            # wqb (wkmC/wk) is needed only after AR#1 (~100us in) — load it
            # after the keys stream.
            nc.sync.dma_start(wqb_sb[:], wqb_d[:, :, :])

            st_t, csss_t = [], []
            for k in range(4):
                stt = sbuf_s.tile([128, 2, VO], bf16, tag="st", bufs=4, name=f"st{k}")
                nc.sync.dma_start(stt[:], st_d[k, :, :, :])
                st_t.append(stt)
                for e in range(2):
                    sb = 2 * k + e
                    csss = sbuf_s.tile([128, 2, HALF], bf16, tag="csss", bufs=4,
                                       name=f"csss{sb}")
                    nc.sync.dma_start(csss[:], csss_d[sb, :, :, :])
                    csss_t.append(csss)

            wep_sb = wts.tile([128, 3, 4096], fp8, tag="wep")
            nc.sync.dma_start(wep_sb[:], wep_d[:, :, :])

            # weight slice helpers (flat fp8 packs)
            wqT_s = lambda kc, lo, hi: wqa_sb[:, 0, kc * 256 + lo : kc * 256 + hi]
            wqm_s = lambda n2, lo, hi: wqa_sb[:, 1, n2 * 2048 + lo : n2 * 2048 + hi]
            wkm_s = lambda h, lo, hi: wqb_sb[:, 0, h * 256 + lo : h * 256 + hi]
            wk_s = lambda jc, lo, hi: wqb_sb[:, 1, jc * 2048 + lo : jc * 2048 + hi]
            wvT_s = lambda ic, lo, hi: wep_sb[:, 0, ic * 256 + lo : ic * 256 + hi]
            wvm_s = lambda jc, lo, hi: wep_sb[:, 1, jc * 2048 + lo : jc * 2048 + hi]
            woT_s = lambda h, lo, hi: wep_sb[:, 2, h * 256 + lo : h * 256 + hi]

            # ================= q path =================
            for nc2 in range(2):
                qt_ps2 = psB.tile([128, 1], f32, tag="pB", name=f"qt_ps2_{nc2}")
                for kc in range(16):
                    nc.tensor.matmul(qt_ps2[:], wqT_s(kc, nc2 * 128, (nc2 + 1) * 128),
                                     x_sb[:, kc : kc + 1], start=(kc == 0), stop=(kc == 15))
                nc.scalar.activation(qT_sb[:, nc2 : nc2 + 1], qt_ps2[:], AF.Copy)

            qhT_ps = psB.tile([128, NUM_HEADS], f32, tag="pB")
            for h in range(NUM_HEADS):
                for nc2 in range(2):
                    nc.tensor.matmul(qhT_ps[:, h : h + 1],
                                     wqm_s(nc2, h * 128, (h + 1) * 128),
                                     qT_sb[:, nc2 : nc2 + 1],
                                     start=(nc2 == 0), stop=(nc2 == 1))
            nc.scalar.activation(qhTp_sb[:], qhT_ps[:], AF.Copy)
            nc.gpsimd.dma_start(bqh_in[:], qhTp_sb[:])
            nc.gpsimd.collective_compute(
                "AllReduce", ALU.add, ins=[bqh_in[:].opt()], outs=[bqh_out[:].opt()],
                replica_groups=RG)
            nc.gpsimd.dma_start(qhT_sb[:], bqh_out[:, :])

            # ================= keys rope (classic, on DVE, pre-rendezvous) ==
            # then cast to fp8 pair-tiles on the idle scalar engine:
            # kf8[g][k] holds tiles (2k, 2k+1) of the j-half g, paired on dim1
            kf8_t = [[], []]
            for ci in range(8):
                a = kab_t[ci][:, 0, :]
                b = kab_t[ci][:, 1, :]
                c = cksk_t[ci][:, 0, :]
                s = cksk_t[ci][:, 1, :]
                t1 = tmps.tile([128, S_LOC], bf16, tag="rt", bufs=8)
                t2 = tmps.tile([128, S_LOC], bf16, tag="rt", bufs=8)
                t3 = tmps.tile([128, S_LOC], bf16, tag="rt", bufs=8)
                t4 = tmps.tile([128, S_LOC], bf16, tag="rt", bufs=8)
                nc.vector.tensor_mul(t1[:], a, c)
                nc.vector.tensor_mul(t2[:], b, s)
                nc.vector.tensor_mul(t3[:], b, c)
                nc.vector.tensor_mul(t4[:], a, s)
                nc.vector.tensor_sub(a, t1[:], t2[:])
                nc.vector.tensor_add(b, t3[:], t4[:])
                k, e = ci // 2, ci % 2
                for g in range(2):
                    if e == 0:
                        kf8_t[g].append(kbuf.tile([128, 2, S_LOC], fp8,
                                                  tag=f"kf8_{g}", bufs=4,
                                                  name=f"kf8_{g}_{k}"))
                    nc.scalar.activation(kf8_t[g][k][:, e, :], kab_t[ci][:, g, :],
                                         AF.Copy)

            # ================= states rope (classic, pre-rendezvous) ========
            # then cast each roped pair-tile to fp8 on the scalar engine
            stf8_t = []
            for k in range(4):
                stt = st_t[k]
                for e in range(2):
                    sb = 2 * k + e
                    c = csss_t[sb][:, 0, :]
                    s = csss_t[sb][:, 1, :]
                    t1 = tmps.tile([128, HALF], bf16, tag="rt", bufs=8)
                    t2 = tmps.tile([128, HALF], bf16, tag="rt", bufs=8)
                    t3 = tmps.tile([128, HALF], bf16, tag="rt", bufs=8)
                    t4 = tmps.tile([128, HALF], bf16, tag="rt", bufs=8)
                    nc.vector.tensor_mul(t1[:], stt[:, e, 0:HALF], c)
                    nc.vector.tensor_mul(t2[:], stt[:, e, HALF:VO], s)
                    nc.vector.tensor_mul(t3[:], stt[:, e, HALF:VO], c)
                    nc.vector.tensor_mul(t4[:], stt[:, e, 0:HALF], s)
                    nc.vector.tensor_sub(stt[:, e, 0:HALF], t1[:], t2[:])
                    nc.vector.tensor_add(stt[:, e, HALF:VO], t3[:], t4[:])
                stf8 = sbuf_s.tile([128, 2, VO], fp8, tag="stf8", bufs=4,
                                   name=f"stf8_{k}")
                nc.scalar.activation(stf8[:].rearrange("p e j -> p (e j)"),
                                     stt[:].rearrange("p e j -> p (e j)"), AF.Copy)
                stf8_t.append(stf8)

            # ================= q path stage 2 (post-AR#1) =================
            tmpT_ps = [psB.tile([128, NUM_HEADS], f32, tag="pB", name=f"tmpT_ps{j}")
                       for j in range(2)]
            for h in range(NUM_HEADS):
                for jc in range(2):
                    nc.tensor.matmul(tmpT_ps[jc][:, h : h + 1],
                                     wkm_s(h, jc * 128, (jc + 1) * 128),
                                     qhT_sb[:, h : h + 1], start=True, stop=True)
            for jc in range(2):
                nc.scalar.activation(tmpT_sb[:, jc, :], tmpT_ps[jc][:], AF.Copy)

            # qtil partials: two halves, each with its own AllReduce so the
            # second mesh pipelines behind the first while logits half 0 runs
            qt_big = [psB.tile([128, 128], f32, tag="pB", name=f"qt_big{g}")
                      for g in range(2)]
            for ic in range(16):
                g, col = ic // 8, (ic % 8) * 16
                for jc in range(2):
                    nc.tensor.matmul(qt_big[g][:, col : col + 16],
                                     wk_s(jc, ic * 128, (ic + 1) * 128),
                                     tmpT_sb[:, jc, :], start=(jc == 0), stop=(jc == 1))
            for g in range(2):
                nc.scalar.activation(qtp_sb[:, g * 128 : (g + 1) * 128], qt_big[g][:],
                                     AF.Copy, scale=SCALE_QTP)
                nc.gpsimd.dma_start(bqt_in[g][:, :], qtp_sb[:, g * 128 : (g + 1) * 128])
                nc.gpsimd.collective_compute(
                    "AllReduce", ALU.add, ins=[bqt_in[g][:].opt()],
                    outs=[bqt_out[g][:].opt()], replica_groups=RG)
                nc.gpsimd.dma_start(
                    qtilT8[g][:],
                    bqt_out[g][:, :].rearrange("p (k e h) -> p k e h", k=4, e=2))

            # logits: 16 DoubleRow matmuls into two PSUM chunks
            lg_ps = [psA.tile([NUM_HEADS, 512], f32, tag="pA", name=f"lg{sc}")
                     for sc in range(2)]
            DR = mybir.MatmulPerfMode.DoubleRow
            for g in range(2):       # half 0 runs while AR#2b is still in flight
                for k in range(4):
                    for sc in range(2):
                        nc.tensor.matmul(
                            lg_ps[sc][:], qtilT8[g][:, k, :, :],
                            kf8_t[g][k][:, :, sc * 512 : (sc + 1) * 512],
                            start=(g == 0 and k == 0), stop=(g == 1 and k == 3),
                            perf_mode=DR)

            # exp + l, interleaved with wT transposes (PE); wT goes to fp8
            # pair-layout (scale 1/16, cancelled by wvT's x16 at the z stage)
            wt_ps = [psB.tile([128, 4 * NUM_HEADS], bf16, tag="pB", name=f"wt_ps{g}")
                     for g in range(2)]
            for sc in range(2):
                nc.scalar.activation(w_sb[:, sc * 512 : (sc + 1) * 512], lg_ps[sc][:],
                                     AF.Exp, scale=SCALE_EXP,
                                     accum_out=(l0_sb[:] if sc == 0 else l1_sb[:]))
                for k in range(4):
                    sb = sc * 4 + k
                    nc.tensor.transpose(wt_ps[sc][:, k * 16 : (k + 1) * 16],
                                        w_sb[:, sb * 128 : (sb + 1) * 128],
                                        identb[0:16, 0:16])
                nc.scalar.activation(
                    wT4_sb[:, 2 * sc : 2 * (sc + 1), :, :]
                    .rearrange("p a e h -> p (a e h)"),
                    wt_ps[sc][:], AF.Copy, scale=float(1.0 / WSCALE))
            nc.vector.tensor_add(lp_sb[:], l0_sb[:], l1_sb[:])

            # u: 16 DoubleRow matmuls into four PSUM chunks
            u_ps = [psA.tile([NUM_HEADS, 512], f32, tag="pA", name=f"u_ps{i}")
                    for i in range(4)]
            for k in range(4):
                for nch in range(4):
                    nc.tensor.matmul(u_ps[nch][:], wT4_sb[:, k, :, :],
                                     stf8_t[k][:, :, nch * 512 : (nch + 1) * 512],
                                     start=(k == 0), stop=(k == 3), perf_mode=DR)
            for nch in range(4):
                if nch % 2 == 0:
                    nc.scalar.activation(u_sb[:, nch * 512 : (nch + 1) * 512],
                                         u_ps[nch][:], AF.Copy)
                else:
                    nc.vector.tensor_copy(u_sb[:, nch * 512 : (nch + 1) * 512],
                                          u_ps[nch][:])

            # uT via PE transposes, bundled into 4 wide copies
            ut_ps = [psB.tile([128, 4 * NUM_HEADS], f32, tag="pB", name=f"ut_ps{g}")
                     for g in range(4)]
            for g in range(4):
                for k in range(4):
                    ic = g * 4 + k
                    nc.tensor.transpose(ut_ps[g][:, k * 16 : (k + 1) * 16],
                                        u_sb[:, ic * 128 : (ic + 1) * 128],
                                        identf[:, :])
                nc.scalar.activation(ub_sb[:, g * 64 : (g + 1) * 64], ut_ps[g][:],
                                     AF.Copy)
            nc.scalar.activation(ub_sb[0:NUM_HEADS, 256:257], lp_sb[:], AF.Copy)
            nc.gpsimd.dma_start(bu_in[:], ub_sb[:])
            nc.gpsimd.collective_compute(
                "AllReduce", ALU.add, ins=[bu_in[:].opt()], outs=[bu_out[:].opt()],
                replica_groups=RG)
            nc.gpsimd.dma_start(
                uT_bf[:], bu_out[:, 0:256].rearrange("p (ic h) -> p ic h", ic=16))
            nc.gpsimd.dma_start(l_sb[:], bu_out[0:NUM_HEADS, 256:257])
            # u carries 1/16 (fp8 w) which cancels wvT's x16 — rl is just 1/l
            nc.vector.tensor_scalar_mul(l16_sb[:], l_sb[:], 1.0)
            nc.vector.reciprocal(rl_sb[:], l16_sb[:])

            # ================= epilogue =================
            z_ps = psB.tile([NUM_HEADS, SH], f32, tag="pB")
            for ic in range(16):
                nc.tensor.matmul(z_ps[:], uT_bf[:, ic, :], wvT_s(ic, 0, 256),
                                 start=(ic == 0), stop=(ic == 15))
            nc.scalar.activation(z_sb[:], z_ps[:], AF.Copy, scale=rl_sb[:])

            zt_ps = psB.tile([128, 2 * NUM_HEADS], bf16, tag="pB")
            for jc in range(2):
                nc.tensor.transpose(zt_ps[:, jc * 16 : (jc + 1) * 16],
                                    z_sb[:, jc * 128 : (jc + 1) * 128],
                                    identb[0:16, 0:16])
            nc.scalar.activation(zT_sb[:].rearrange("p a h -> p (a h)"), zt_ps[:],
                                 AF.Copy)

            at_ps = psB.tile([128, NUM_HEADS], f32, tag="pB")
            for h in range(NUM_HEADS):
                for jc in range(2):
                    nc.tensor.matmul(at_ps[:, h : h + 1],
                                     wvm_s(jc, h * 128, (h + 1) * 128),
                                     zT_sb[:, jc, h : h + 1],
                                     start=(jc == 0), stop=(jc == 1))
            # 1/256 descales wvm's x16 and pre-compensates woT's x16
            nc.scalar.activation(atT_sb[:], at_ps[:], AF.Copy,
                                 scale=float(1.0 / (WSCALE * WSCALE)))
            # transpose to [16, 128] for the partition-axis AllGather
            atr_ps = psB.tile([NUM_HEADS, 128], bf16, tag="pB")
            nc.tensor.transpose(atr_ps[:], atT_sb[:], identb[:, :])
            nc.scalar.activation(atr_sb[:], atr_ps[:], AF.Copy)
            nc.gpsimd.dma_start(bat_in[:], atr_sb[:])
            nc.gpsimd.collective_compute(
                "AllGather", ALU.bypass, ins=[bat_in[:].opt()], outs=[bat_out[:].opt()],
                replica_groups=RG)
            nc.gpsimd.dma_start(aga_sb[:], bat_out[:, :])
            # transpose the gathered [16r+h, d] blocks to [d, 16r+h], then the
            # 8 rank blocks become free-dim slices we can sum on DVE
            agat_ps = psB.tile([128, 128], bf16, tag="pB")
            nc.tensor.transpose(agat_ps[:], aga_sb[:], identb[:, :])
            nc.scalar.activation(agat_sb[:], agat_ps[:], AF.Copy)
            nc.vector.tensor_add(atT_bf[:], agat_sb[:, 0:16], agat_sb[:, 16:32])
            for r in range(2, 8):
                nc.vector.tensor_add(atT_bf[:], atT_bf[:],
                                     agat_sb[:, 16 * r : 16 * (r + 1)])

            o_ps = psB.tile([1, SH], f32, tag="pB")
            for h in range(NUM_HEADS):
                nc.tensor.matmul(o_ps[:], atT_bf[:, h : h + 1], woT_s(h, 0, 256),
                                 start=(h == 0), stop=(h == NUM_HEADS - 1))
            nc.vector.tensor_add(out_sb[:], o_ps[:], xo_sb[:])
            nc.sync.dma_start(out_d[:, :], out_sb[:])

    nc.compile()
    return nc


def _tables():
    half = HALF
    freqs = 1.0 / (ROPE_THETA ** (np.arange(half, dtype=np.float32) * 2.0 / VO))
    ang = np.outer(np.arange(S, dtype=np.float32), freqs).astype(np.float32)  # (S, half)
    return np.cos(ang), np.sin(ang)


def _w8flat(a, tiles):
    # [tiles*128, X] fp8-prescaled -> [128, tiles*X] (partition-contiguous)
    X = a.shape[1]
    sw = np.ascontiguousarray(
        (np.asarray(a, np.float32) * WSCALE).reshape(tiles, 128, X).transpose(1, 0, 2))
    return sw.reshape(128, tiles * X).astype(FP8)


def kernel(x, keys, states, Wq, Wk, Wv, Wq_mha, Wk_mha, Wv_mha, Wo):
    from concourse import bass_utils

    if "nc" not in _cache:
        _cache["nc"] = _build()
    nc = _cache["nc"]

    x = np.asarray(x, np.float32)
    keys = np.asarray(keys, np.float32)
    states = np.asarray(states, np.float32)
    cos_t, sin_t = _tables()

    xq2d = np.ascontiguousarray(x.reshape(16, 128).T).astype(BF16)
    ident128b = np.eye(128, dtype=np.float32).astype(BF16)
    ident16f = np.eye(16, dtype=np.float32)

    in_maps = []
    for c in range(NC):
        rs = slice(c * SH, (c + 1) * SH)
        ssl = slice(c * S_LOC, (c + 1) * S_LOC)
        cosc = cos_t[ssl]            # (1024, 1024) [s_loc, j]
        sinc = sin_t[ssl]

        wqa = np.stack([
            _w8flat(Wq[rs].T, 16),
            _w8flat(Wq_mha[:, rs].T, 2),
        ], axis=1)                   # [128, 2, 4096]
        wqb = np.stack([
            _w8flat(Wk_mha[:, rs], 16),
            _w8flat(Wk[rs], 2),
        ], axis=1)                   # [128, 2, 4096]
        wep = np.stack([
            _w8flat(Wv[rs].T, 16),
            _w8flat(Wv_mha[:, rs].T, 2),
            _w8flat(Wo[rs].T, 16),
        ], axis=1)                   # [128, 3, 4096]

        kT = keys[ssl].T.astype(BF16).reshape(16, 128, S_LOC)
        kab = np.stack([kT[0:8], kT[8:16]], axis=2)          # [8, 128, 2, 1024]
        cT = cosc.T.astype(BF16).reshape(8, 128, S_LOC)
        sT = sinc.T.astype(BF16).reshape(8, 128, S_LOC)
        cksk = np.stack([cT, sT], axis=2)                    # [8, 128, 2, 1024]
        cN = cosc.astype(BF16).reshape(8, 128, HALF)
        sN = sinc.astype(BF16).reshape(8, 128, HALF)
        csss = np.stack([cN, sN], axis=2)                    # [8, 128, 2, 1024]

        m = {
            "xq": xq2d,
            "identb": ident128b,
            "identf": ident16f,
            "xo": np.ascontiguousarray(x[rs]).reshape(1, SH),
            "wqa": np.ascontiguousarray(wqa),
            "wqb": np.ascontiguousarray(wqb),
            "wep": np.ascontiguousarray(wep),
            "kab": np.ascontiguousarray(kab),
            "cksk": np.ascontiguousarray(cksk),
            "st": np.ascontiguousarray(
                states[ssl].astype(BF16).reshape(4, 2, 128, VO)
                .transpose(0, 2, 1, 3)),
            "csss": np.ascontiguousarray(csss),
        }
        in_maps.append(m)

    global _last_in_maps, _last_res
    _last_in_maps = in_maps
    res = bass_utils.run_bass_kernel_spmd(nc, in_maps, core_ids=list(range(NC)))
    _last_res = res
    out = np.concatenate([np.asarray(res.results[c]["out"]).reshape(-1) for c in range(NC)])
    return out[None, :].astype(np.float32)
